# revision 13
# baseline (speedup 1.0000x reference)
"""Masked-softmax attention-scores kernel for Trainium2 (Bass/Tile), 8 cores.

Computes softmax(mask_fill(QK^T/sqrt(dk)) + syntax) for
q = query @ Wq.T + bq, k = key @ Wk.T + bk, heads split from d_model.

Sharding: 8 cores = 2 batches x 4 query-row quarters; every core handles all
12 heads for its (batch, row-slice).  The host passes query/key/W already
transposed (pure layout prep), so the device kernel is:
  - project q rows + full key into head-transposed qT/kT [d_model x s]
    (f32r matmuls, fp32 accumulate; 1/sqrt(dk) folded into the qT copy),
  - per 128-row tile: comb = (mask*1e9 - 1e9) + syntax on GPSIMD,
  - per head: scores matmul (K=64) into PSUM, DVE adds comb, ACT exp with
    fused row-sum, DVE reciprocal, normalize (DVE/ACT split), DMA out.
Softmax is computed without max-subtraction: scores + syntax are O(10) here
(exp cannot overflow) and masked entries sit at ~-1e9 whose exp underflows
to exactly 0, matching the reference's -1e9 mask fill.
"""

from contextlib import ExitStack

import numpy as np

B, S, D, H = 2, 2048, 768, 12
DK = D // H
P = 128
NCORES = 8
RSPLIT = 4          # query-row splits per batch
R = S // RSPLIT     # query rows per core
NEG = -1.0e9


def build_program(S=S, D=D, H=H, R=R, mm_f32r=True, mul_act_every=3):
    """Build the per-core SPMD Bass program (same program, 8 data shards)."""
    import concourse.bacc as bacc
    import concourse.mybir as mybir
    from concourse.tile import TileContext

    f32 = mybir.dt.float32
    i32 = mybir.dt.int32
    MMDT = mybir.dt.float32r if mm_f32r else f32
    ADD = mybir.AluOpType.add
    MULT = mybir.AluOpType.mult
    EXP = mybir.ActivationFunctionType.Exp
    COPY = mybir.ActivationFunctionType.Copy

    assert D % P == 0 and S % 512 == 0 and R % P == 0 and D // H == 64
    DC = D // P      # d-model chunks (6)
    RT = R // P      # query row tiles per core (4)
    NB = S // 512    # key-position blocks (4)

    nc = bacc.Bacc(trn_type="TRN2", target_bir_lowering=False, debug=False)

    # Host passes qt/kt/w*t pre-transposed (feature dim leading).
    qt_in = nc.declare_dram_parameter("qt_in", [D, R], MMDT, isOutput=False)
    kt_in = nc.declare_dram_parameter("kt_in", [D, S], MMDT, isOutput=False)
    syn = nc.declare_dram_parameter("syn", [R, S], f32, isOutput=False)
    msk = nc.declare_dram_parameter("msk", [R, S], i32, isOutput=False)
    wqt = nc.declare_dram_parameter("wqt", [D, D], MMDT, isOutput=False)
    bq = nc.declare_dram_parameter("bq", [D], f32, isOutput=False)
    wkt = nc.declare_dram_parameter("wkt", [D, D], MMDT, isOutput=False)
    bk = nc.declare_dram_parameter("bk", [D], f32, isOutput=False)
    out = nc.declare_dram_parameter("out", [H, R, S], f32, isOutput=True)

    with ExitStack() as ctx:
        tc = ctx.enter_context(TileContext(nc))

        consts = ctx.enter_context(tc.tile_pool(name="consts", bufs=1))
        ones_f = consts.tile([1, 512], f32)
        nc.vector.memset(ones_f, 1.0)
        ones = consts.tile([1, 512], MMDT)
        nc.vector.tensor_copy(ones, ones_f)

        # Persistent projected activations, head-transposed: qT/kT[dc] holds
        # d_model rows [dc*128, dc*128+128) x all s columns.
        persist = ctx.enter_context(tc.tile_pool(name="persist", bufs=1))
        kT = [persist.tile([P, S], MMDT, name=f"kT{i}", tag=f"kT{i}")
              for i in range(DC)]
        qT = [persist.tile([P, R], MMDT, name=f"qT{i}", tag=f"qT{i}")
              for i in range(DC)]

        # ---------------- prep (scoped: freed before the main loop) -------
        with (
            tc.tile_pool(name="wprep", bufs=1) as wpool,
            tc.tile_pool(name="xprep", bufs=1) as xpool,
            tc.tile_pool(name="pproj", bufs=4, space="PSUM") as proj_pool,
        ):
            wqT = [wpool.tile([P, D], MMDT, name=f"wqT{i}", tag=f"wqT{i}")
                   for i in range(DC)]
            wkT = [wpool.tile([P, D], MMDT, name=f"wkT{i}", tag=f"wkT{i}")
                   for i in range(DC)]
            bq_f = wpool.tile([1, D], f32, tag="bq_f")
            bk_f = wpool.tile([1, D], f32, tag="bk_f")
            bqs = wpool.tile([1, D], MMDT, tag="bqs")
            bks = wpool.tile([1, D], MMDT, tag="bks")
            nc.sync.dma_start(out=bq_f, in_=bq[None, :])
            nc.sync.dma_start(out=bk_f, in_=bk[None, :])
            nc.vector.tensor_copy(bqs, bq_f)
            nc.vector.tensor_copy(bks, bk_f)

            qTraw = [xpool.tile([P, R], MMDT, name=f"qTraw{i}", tag=f"qTraw{i}")
                     for i in range(DC)]
            kTraw = [xpool.tile([P, S], MMDT, name=f"kTraw{i}", tag=f"kTraw{i}")
                     for i in range(DC)]
            for fj in range(DC):
                nc.sync.dma_start(out=wqT[fj], in_=wqt[fj * P:(fj + 1) * P, :])
                nc.sync.dma_start(out=wkT[fj], in_=wkt[fj * P:(fj + 1) * P, :])
                nc.sync.dma_start(out=qTraw[fj], in_=qt_in[fj * P:(fj + 1) * P, :])
            for nb in range(NB):
                for fj in range(DC):
                    cols = slice(nb * 512, (nb + 1) * 512)
                    nc.sync.dma_start(out=kTraw[fj][:, cols],
                                      in_=kt_in[fj * P:(fj + 1) * P, cols])

            # q projection: qT[dm] = (Wq @ queryT + bq) / 8
            for dm in range(DC):
                for rb in range(max(1, R // 512)):
                    rw = min(512, R)
                    sl = slice(rb * 512, rb * 512 + rw)
                    ps = proj_pool.tile([P, rw], f32, tag="psq")
                    for fj in range(DC):
                        nc.tensor.matmul(
                            ps, wqT[fj][:, dm * P:(dm + 1) * P], qTraw[fj][:, sl],
                            start=(fj == 0), stop=False,
                        )
                    nc.tensor.matmul(
                        ps, bqs[0:1, dm * P:(dm + 1) * P], ones[0:1, :rw],
                        start=False, stop=True,
                    )
                    nc.scalar.activation(qT[dm][:, sl], ps, COPY, bias=0.0,
                                         scale=1.0 / 8.0)

            # k projection: kT[dm] = Wk @ keyT + bk  (nb-outer: start on
            # each 512-column block as soon as its DMA chunks land)
            for nb in range(NB):
                for dm in range(DC):
                    cols = slice(nb * 512, (nb + 1) * 512)
                    ps = proj_pool.tile([P, 512], f32, tag="psk")
                    for fj in range(DC):
                        nc.tensor.matmul(
                            ps, wkT[fj][:, dm * P:(dm + 1) * P], kTraw[fj][:, cols],
                            start=(fj == 0), stop=False,
                        )
                    nc.tensor.matmul(
                        ps, bks[0:1, dm * P:(dm + 1) * P], ones,
                        start=False, stop=True,
                    )
                    nc.scalar.activation(kT[dm][:, cols], ps, COPY, bias=0.0,
                                         scale=1.0)

        # ---------------- main loop: scores + masked softmax --------------
        mskp = ctx.enter_context(tc.tile_pool(name="mskp", bufs=2))
        synp = ctx.enter_context(tc.tile_pool(name="synp", bufs=2))
        combp = ctx.enter_context(tc.tile_pool(name="combp", bufs=2))
        spool = ctx.enter_context(tc.tile_pool(name="spool", bufs=2))
        epool = ctx.enter_context(tc.tile_pool(name="epool", bufs=2))
        opool = ctx.enter_context(tc.tile_pool(name="opool", bufs=2))
        rpool = ctx.enter_context(tc.tile_pool(name="rpool", bufs=8))
        pspool = ctx.enter_context(tc.tile_pool(name="pspool", bufs=2, space="PSUM"))

        for t in range(RT):
            rows = slice(t * P, (t + 1) * P)
            # comb = (mask ? 0 : -1e9) + syntax   (gpsimd; DVE is loaded)
            msk_t = mskp.tile([P, S], i32)
            nc.sync.dma_start(out=msk_t, in_=msk[rows, :])
            syn_t = synp.tile([P, S], f32)
            nc.sync.dma_start(out=syn_t, in_=syn[rows, :])
            comb = combp.tile([P, S], f32)
            nc.gpsimd.tensor_scalar(comb, msk_t, 1.0e9, NEG, op0=MULT, op1=ADD)
            nc.gpsimd.tensor_add(comb, comb, syn_t)

            for h in range(H):
                dc, off = h // 2, 64 * (h % 2)
                ps = pspool.tile([P, S], f32)
                for nb in range(NB):
                    cols = slice(nb * 512, (nb + 1) * 512)
                    nc.tensor.matmul(
                        ps[:, cols],
                        qT[dc][off:off + 64, rows],
                        kT[dc][off:off + 64, cols],
                        start=True, stop=True,
                    )
                s_t = spool.tile([P, S], f32, tag="s")
                nc.vector.tensor_add(s_t, ps, comb)
                e = epool.tile([P, S], f32)
                rowsum = rpool.tile([P, 1], f32)
                nc.scalar.activation(e, s_t, EXP, accum_out=rowsum)
                rrec = rpool.tile([P, 1], f32)
                nc.vector.reciprocal(rrec, rowsum)
                o = opool.tile([P, S], f32)
                nc.gpsimd.tensor_scalar(o, e, scalar1=rrec, scalar2=None,
                                        op0=MULT)
                nc.sync.dma_start(out=out[h, rows, :], in_=o)

    nc.finalize()
    return nc


_NC_CACHE = {}


def _get_nc():
    key = "full"
    if key not in _NC_CACHE:
        _NC_CACHE[key] = build_program()
    return _NC_CACHE[key]


def shard_inputs(query, key, syntax_matrix, mask, Wq, bq, Wk, bk):
    wqt = np.ascontiguousarray(Wq.T, np.float32)
    wkt = np.ascontiguousarray(Wk.T, np.float32)
    bq = np.ascontiguousarray(bq, np.float32)
    bk = np.ascontiguousarray(bk, np.float32)
    in_maps = []
    for c in range(NCORES):
        b, r = divmod(c, RSPLIT)
        rows = slice(r * R, (r + 1) * R)
        in_maps.append({
            "qt_in": np.ascontiguousarray(query[b, rows, :].T, np.float32),
            "kt_in": np.ascontiguousarray(key[b].T, np.float32),
            "syn": np.ascontiguousarray(syntax_matrix[b, 0, rows, :], np.float32),
            "msk": np.ascontiguousarray(mask[b, rows, :], np.int32),
            "wqt": wqt,
            "bq": bq,
            "wkt": wkt,
            "bk": bk,
        })
    return in_maps


def assemble_output(results):
    out = np.empty((B, H, S, S), np.float32)
    for c in range(NCORES):
        b, r = divmod(c, RSPLIT)
        out[b, :, r * R:(r + 1) * R, :] = results[c]["out"]
    return out


def run_spmd(in_maps, **kwargs):
    from concourse.bass_utils import run_bass_kernel_spmd

    nc = _get_nc()
    return run_bass_kernel_spmd(nc, in_maps, list(range(NCORES)), **kwargs)


def kernel(query, key, vm, syntax_matrix, mask, Wq, bq, Wk, bk):
    query = np.asarray(query, np.float32)
    key = np.asarray(key, np.float32)
    syntax_matrix = np.asarray(syntax_matrix, np.float32)
    mask = np.asarray(mask, np.int32)
    Wq = np.asarray(Wq, np.float32)
    bq = np.asarray(bq, np.float32)
    Wk = np.asarray(Wk, np.float32)
    bk = np.asarray(bk, np.float32)

    in_maps = shard_inputs(query, key, syntax_matrix, mask, Wq, bq, Wk, bk)
    res = run_spmd(in_maps)
    return assemble_output(res.results)


# revision 14
# speedup vs baseline: 4.5545x; 4.5545x over previous
"""Masked-softmax attention-scores kernel for Trainium2 (Bass/Tile), 8 cores.

Computes softmax(mask_fill(QK^T/sqrt(dk)) + syntax) for
q = query @ Wq.T + bq, k = key @ Wk.T + bk, heads split from d_model.

Sharding: 8 cores = 2 batches x 4 query-row quarters; every core handles all
12 heads for its (batch, row-slice).  The host passes query/key/W already
transposed (pure layout prep), so the device kernel is:
  - project q rows + full key into head-transposed qT/kT [d_model x s]
    (f32r matmuls, fp32 accumulate; 1/sqrt(dk) folded into the qT copy),
  - per 128-row tile: comb = (mask*1e9 - 1e9) + syntax on GPSIMD,
  - per head: scores matmul (K=64) into PSUM, DVE adds comb, ACT exp with
    fused row-sum, DVE reciprocal, normalize (DVE/ACT split), DMA out.
Softmax is computed without max-subtraction: scores + syntax are O(10) here
(exp cannot overflow) and masked entries sit at ~-1e9 whose exp underflows
to exactly 0, matching the reference's -1e9 mask fill.
"""

from contextlib import ExitStack

import numpy as np

B, S, D, H = 2, 2048, 768, 12
DK = D // H
P = 128
NCORES = 8
RSPLIT = 4          # query-row splits per batch
R = S // RSPLIT     # query rows per core
NEG = -1.0e9


def build_program(S=S, D=D, H=H, R=R, mm_f32r=True, mul_act_every=3):
    """Build the per-core SPMD Bass program (same program, 8 data shards)."""
    import concourse.bacc as bacc
    import concourse.mybir as mybir
    from concourse.tile import TileContext

    f32 = mybir.dt.float32
    i32 = mybir.dt.int32
    MMDT = mybir.dt.float32r if mm_f32r else f32
    ADD = mybir.AluOpType.add
    MULT = mybir.AluOpType.mult
    EXP = mybir.ActivationFunctionType.Exp
    COPY = mybir.ActivationFunctionType.Copy

    assert D % P == 0 and S % 512 == 0 and R % P == 0 and D // H == 64
    DC = D // P      # d-model chunks (6)
    RT = R // P      # query row tiles per core (4)
    NB = S // 512    # key-position blocks (4)

    nc = bacc.Bacc(trn_type="TRN2", target_bir_lowering=False, debug=False)

    # Host passes qt/kt/w*t pre-transposed (feature dim leading).
    qt_in = nc.declare_dram_parameter("qt_in", [D, R], MMDT, isOutput=False)
    kt_in = nc.declare_dram_parameter("kt_in", [D, S], MMDT, isOutput=False)
    syn = nc.declare_dram_parameter("syn", [R, S], f32, isOutput=False)
    msk = nc.declare_dram_parameter("msk", [R, S], i32, isOutput=False)
    wqt = nc.declare_dram_parameter("wqt", [D, D], MMDT, isOutput=False)
    bq = nc.declare_dram_parameter("bq", [D], f32, isOutput=False)
    wkt = nc.declare_dram_parameter("wkt", [D, D], MMDT, isOutput=False)
    bk = nc.declare_dram_parameter("bk", [D], f32, isOutput=False)
    out = nc.declare_dram_parameter("out", [H, R, S], f32, isOutput=True)

    with ExitStack() as ctx:
        tc = ctx.enter_context(TileContext(nc))

        consts = ctx.enter_context(tc.tile_pool(name="consts", bufs=1))
        ones_f = consts.tile([1, 512], f32)
        nc.vector.memset(ones_f, 1.0)
        ones = consts.tile([1, 512], MMDT)
        nc.vector.tensor_copy(ones, ones_f)

        # Persistent projected activations, head-transposed: qT/kT[dc] holds
        # d_model rows [dc*128, dc*128+128) x all s columns.
        persist = ctx.enter_context(tc.tile_pool(name="persist", bufs=1))
        kT = [persist.tile([P, S], MMDT, name=f"kT{i}", tag=f"kT{i}")
              for i in range(DC)]
        qT = [persist.tile([P, R], MMDT, name=f"qT{i}", tag=f"qT{i}")
              for i in range(DC)]

        # ---------------- prep (scoped: freed before the main loop) -------
        with (
            tc.tile_pool(name="wprep", bufs=1) as wpool,
            tc.tile_pool(name="xprep", bufs=1) as xpool,
            tc.tile_pool(name="pproj", bufs=4, space="PSUM") as proj_pool,
        ):
            wqT = [wpool.tile([P, D], MMDT, name=f"wqT{i}", tag=f"wqT{i}")
                   for i in range(DC)]
            wkT = [wpool.tile([P, D], MMDT, name=f"wkT{i}", tag=f"wkT{i}")
                   for i in range(DC)]
            bq_f = wpool.tile([1, D], f32, tag="bq_f")
            bk_f = wpool.tile([1, D], f32, tag="bk_f")
            bqs = wpool.tile([1, D], MMDT, tag="bqs")
            bks = wpool.tile([1, D], MMDT, tag="bks")
            nc.sync.dma_start(out=bq_f, in_=bq[None, :])
            nc.sync.dma_start(out=bk_f, in_=bk[None, :])
            nc.vector.tensor_copy(bqs, bq_f)
            nc.vector.tensor_copy(bks, bk_f)

            qTraw = [xpool.tile([P, R], MMDT, name=f"qTraw{i}", tag=f"qTraw{i}")
                     for i in range(DC)]
            kTraw = [xpool.tile([P, S], MMDT, name=f"kTraw{i}", tag=f"kTraw{i}")
                     for i in range(DC)]
            for fj in range(DC):
                nc.sync.dma_start(out=wqT[fj], in_=wqt[fj * P:(fj + 1) * P, :])
                nc.sync.dma_start(out=wkT[fj], in_=wkt[fj * P:(fj + 1) * P, :])
                nc.sync.dma_start(out=qTraw[fj], in_=qt_in[fj * P:(fj + 1) * P, :])
            for nb in range(NB):
                for fj in range(DC):
                    cols = slice(nb * 512, (nb + 1) * 512)
                    nc.sync.dma_start(out=kTraw[fj][:, cols],
                                      in_=kt_in[fj * P:(fj + 1) * P, cols])

            # q projection: qT[dm] = (Wq @ queryT + bq) / 8
            for dm in range(DC):
                for rb in range(max(1, R // 512)):
                    rw = min(512, R)
                    sl = slice(rb * 512, rb * 512 + rw)
                    ps = proj_pool.tile([P, rw], f32, tag="psq")
                    for fj in range(DC):
                        nc.tensor.matmul(
                            ps, wqT[fj][:, dm * P:(dm + 1) * P], qTraw[fj][:, sl],
                            start=(fj == 0), stop=False,
                        )
                    nc.tensor.matmul(
                        ps, bqs[0:1, dm * P:(dm + 1) * P], ones[0:1, :rw],
                        start=False, stop=True,
                    )
                    nc.scalar.activation(qT[dm][:, sl], ps, COPY, bias=0.0,
                                         scale=1.0 / 8.0)

            # k projection: kT[dm] = Wk @ keyT + bk  (nb-outer: start on
            # each 512-column block as soon as its DMA chunks land)
            for nb in range(NB):
                for dm in range(DC):
                    cols = slice(nb * 512, (nb + 1) * 512)
                    ps = proj_pool.tile([P, 512], f32, tag="psk")
                    for fj in range(DC):
                        nc.tensor.matmul(
                            ps, wkT[fj][:, dm * P:(dm + 1) * P], kTraw[fj][:, cols],
                            start=(fj == 0), stop=False,
                        )
                    nc.tensor.matmul(
                        ps, bks[0:1, dm * P:(dm + 1) * P], ones,
                        start=False, stop=True,
                    )
                    nc.scalar.activation(kT[dm][:, cols], ps, COPY, bias=0.0,
                                         scale=1.0)

        # ---------------- main loop: scores + masked softmax --------------
        mskp = ctx.enter_context(tc.tile_pool(name="mskp", bufs=2))
        synp = ctx.enter_context(tc.tile_pool(name="synp", bufs=2))
        combp = ctx.enter_context(tc.tile_pool(name="combp", bufs=2))
        spool = ctx.enter_context(tc.tile_pool(name="spool", bufs=2))
        epool = ctx.enter_context(tc.tile_pool(name="epool", bufs=2))
        opool = ctx.enter_context(tc.tile_pool(name="opool", bufs=2))
        rpool = ctx.enter_context(tc.tile_pool(name="rpool", bufs=8))
        pspool = ctx.enter_context(tc.tile_pool(name="pspool", bufs=2, space="PSUM"))

        for t in range(RT):
            rows = slice(t * P, (t + 1) * P)
            # comb = (mask ? 0 : -1e9) + syntax   (gpsimd; DVE is loaded)
            msk_t = mskp.tile([P, S], i32)
            nc.sync.dma_start(out=msk_t, in_=msk[rows, :])
            syn_t = synp.tile([P, S], f32)
            nc.sync.dma_start(out=syn_t, in_=syn[rows, :])
            comb = combp.tile([P, S], f32)
            nc.gpsimd.tensor_scalar(comb, msk_t, 1.0e9, NEG, op0=MULT, op1=ADD)
            nc.gpsimd.tensor_add(comb, comb, syn_t)

            for h in range(H):
                dc, off = h // 2, 64 * (h % 2)
                ps = pspool.tile([P, S], f32)
                for nb in range(NB):
                    cols = slice(nb * 512, (nb + 1) * 512)
                    nc.tensor.matmul(
                        ps[:, cols],
                        qT[dc][off:off + 64, rows],
                        kT[dc][off:off + 64, cols],
                        start=True, stop=True,
                    )
                s_t = spool.tile([P, S], f32, tag="s")
                nc.vector.tensor_add(s_t, ps, comb)
                e = epool.tile([P, S], f32)
                rowsum = rpool.tile([P, 1], f32)
                nc.scalar.activation(e, s_t, EXP, accum_out=rowsum)
                rrec = rpool.tile([P, 1], f32)
                nc.vector.reciprocal(rrec, rowsum)
                o = opool.tile([P, S], f32)
                if h % mul_act_every == mul_act_every - 1:
                    nc.scalar.activation(o, e, COPY, bias=0.0, scale=rrec)
                else:
                    nc.vector.tensor_scalar(o, e, scalar1=rrec, scalar2=None,
                                            op0=MULT)
                nc.sync.dma_start(out=out[h, rows, :], in_=o)

    nc.finalize()
    return nc


_NC_CACHE = {}


def _get_nc():
    key = "full"
    if key not in _NC_CACHE:
        _NC_CACHE[key] = build_program()
    return _NC_CACHE[key]


def shard_inputs(query, key, syntax_matrix, mask, Wq, bq, Wk, bk):
    wqt = np.ascontiguousarray(Wq.T, np.float32)
    wkt = np.ascontiguousarray(Wk.T, np.float32)
    bq = np.ascontiguousarray(bq, np.float32)
    bk = np.ascontiguousarray(bk, np.float32)
    in_maps = []
    for c in range(NCORES):
        b, r = divmod(c, RSPLIT)
        rows = slice(r * R, (r + 1) * R)
        in_maps.append({
            "qt_in": np.ascontiguousarray(query[b, rows, :].T, np.float32),
            "kt_in": np.ascontiguousarray(key[b].T, np.float32),
            "syn": np.ascontiguousarray(syntax_matrix[b, 0, rows, :], np.float32),
            "msk": np.ascontiguousarray(mask[b, rows, :], np.int32),
            "wqt": wqt,
            "bq": bq,
            "wkt": wkt,
            "bk": bk,
        })
    return in_maps


def assemble_output(results):
    out = np.empty((B, H, S, S), np.float32)
    for c in range(NCORES):
        b, r = divmod(c, RSPLIT)
        out[b, :, r * R:(r + 1) * R, :] = results[c]["out"]
    return out


def run_spmd(in_maps, **kwargs):
    from concourse.bass_utils import run_bass_kernel_spmd

    nc = _get_nc()
    return run_bass_kernel_spmd(nc, in_maps, list(range(NCORES)), **kwargs)


def kernel(query, key, vm, syntax_matrix, mask, Wq, bq, Wk, bk):
    query = np.asarray(query, np.float32)
    key = np.asarray(key, np.float32)
    syntax_matrix = np.asarray(syntax_matrix, np.float32)
    mask = np.asarray(mask, np.int32)
    Wq = np.asarray(Wq, np.float32)
    bq = np.asarray(bq, np.float32)
    Wk = np.asarray(Wk, np.float32)
    bk = np.asarray(bk, np.float32)

    in_maps = shard_inputs(query, key, syntax_matrix, mask, Wq, bq, Wk, bk)
    res = run_spmd(in_maps)
    return assemble_output(res.results)
